# revision 5
# baseline (speedup 1.0000x reference)
"""Energy Transformer descent kernel for Trainium2 (8 NeuronCores).

Data-parallel over batch B=8: one batch element per core. Per core, 12 descent
steps run on-chip in a For_i loop; x stays resident in SBUF.

Per step (all matmuls fp32r = full-rate PE with ~1.8e-4 relative precision):
  g = LayerNorm(x)       (bn_stats/bn_aggr; rstd via exp(-0.5*ln(var+eps)))
  gT via PE transpose
  Per head: Qt/Kt = W @ gT (heads-stacked), S_qk per q-tile -> exp(beta*S)
  with fused row-sum Z (ACT accum_out); A_kT accumulates (Q/Z)^T @ E_qk.
  lnZ row (PE transpose + cross-partition DMA) augments Qhat row 64 so the
  65-row contraction S_kq = beta*K.Q - lnZ gives P = softmax directly;
  A_qT accumulates K^T @ P.  grad_att = [A_qT;A_kT]^T @ alpha*[Wq;Wk].
  Hopfield: H^T = (gamma*xi)^T... per m-tile -> relu -> RT; grad accumulates
  RT^T @ alpha*xi.  x += grad (alpha folded into wnat/xin on host).
gamma is folded into the D-rows of wqkt/xit on the host; beta_ln must be 0.
"""
import sys
sys.path.insert(0, "/opt/trn_rl_repo")
import functools
import numpy as np

B, N, D = 8, 1024, 768
H, Y, M = 12, 64, 3072
STEPS = 12
ALPHA = 0.1
EPS = 1e-5
BETA = 1.0 / float(np.sqrt(Y))
ESHIFT = 30.0  # exp(beta*S - ESHIFT); cancels in softmax, guards overflow
NT, DT, MT = N // 128, D // 128, M // 128   # 8, 6, 24
HPAIRS = H // 2                              # 6
MG, MGN = 8, 3                               # hopfield: 8 groups of 3 m-tiles


def _build(steps=STEPS):
    import concourse.bacc as bacc
    import concourse.mybir as mybir
    import concourse.tile as tile
    from concourse.masks import make_identity

    F32 = mybir.dt.float32
    F32R = mybir.dt.float32r
    AF = mybir.ActivationFunctionType

    import concourse.tile_utils as tile_utils
    tile_utils.max_sbuf_usage = 206 * 1024  # 224 phys / 208 usable on trn2
    nc = bacc.Bacc(None, debug=False)
    X = nc.declare_dram_parameter("x", [N, D], F32, isOutput=False)
    WQKT = nc.declare_dram_parameter("wqkt", [D, H * 128], F32R, isOutput=False)
    WNAT = nc.declare_dram_parameter("wnat", [H, 128, D], F32R, isOutput=False)
    XIT = nc.declare_dram_parameter("xit", [D, M], F32R, isOutput=False)
    XIN = nc.declare_dram_parameter("xin", [M, D], F32R, isOutput=False)
    OUT = nc.declare_dram_parameter("out", [N, D], F32, isOutput=True)

    wqkt_d = WQKT[:, :].rearrange("(t p) c -> p t c", p=128)   # [128, DT, 1536]
    xit_d = XIT[:, :].rearrange("(t p) m -> p t m", p=128)     # [128, DT, M]
    xin_d = XIN[:, :].rearrange("(t p) d -> p t d", p=128)     # [128, MT, D]
    x_d = X[:, :].rearrange("(t p) d -> p t d", p=128)         # [128, NT, D]
    out_d = OUT[:, :].rearrange("(t p) d -> p t d", p=128)

    with tile.TileContext(nc) as tc:
        with (
            tc.tile_pool(name="persist", bufs=1) as pers,
            tc.tile_pool(name="stream", bufs=2) as st,
            tc.tile_pool(name="st1", bufs=1) as st1,
            tc.tile_pool(name="xitp", bufs=3) as xitp,
            tc.tile_pool(name="heads", bufs=2) as hd,
            tc.tile_pool(name="small", bufs=4) as sm,
            tc.tile_pool(name="psb", bufs=4, space="PSUM") as psb,
            tc.tile_pool(name="psa", bufs=2, space="PSUM") as psa,
            tc.tile_pool(name="psg", bufs=2, space="PSUM") as psg,
        ):
            x_sb = pers.tile([128, NT, D], F32)
            gT = pers.tile([128, DT, N], F32R)
            ident = pers.tile([128, 128], F32)
            eps_t = pers.tile([128, 1], F32)
            negc_t = pers.tile([128, 1], F32)
            ones_row = pers.tile([1, N], F32)
            qkn = pers.tile([128, NT, 256], F32R)
            ast = pers.tile([128, 2, N], F32R)

            ones_c = pers.tile([1, 64], F32R)
            make_identity(nc, ident)
            nc.vector.memset(eps_t, EPS)
            nc.vector.memset(negc_t, -ESHIFT)
            nc.vector.memset(ones_row, 1.0)
            nc.scalar.copy(out=ones_c, in_=ones_row[0:1, 0:64])
            nc.sync.dma_start(out=x_sb, in_=x_d)

            def step_body(_iv=None):
                # ---- Phase A: LayerNorm + transpose into gT ----
                for nt in range(NT):
                    xt = x_sb[:, nt, :]
                    stats = sm.tile([128, 3, 6], F32, name="stats")
                    xg = xt.rearrange("p (s f) -> p s f", s=3)
                    for s in range(3):
                        nc.vector.bn_stats(out=stats[:, s, :], in_=xg[:, s, :])
                    mv = sm.tile([128, 2], F32, name="mv")
                    nc.vector.bn_aggr(out=mv, in_=stats)
                    lnv = sm.tile([128, 1], F32, name="lnv")
                    nc.scalar.activation(out=lnv, in_=mv[:, 1:2], func=AF.Ln,
                                         bias=eps_t)
                    rstd = sm.tile([128, 1], F32, name="rstd")
                    nc.scalar.activation(out=rstd, in_=lnv, func=AF.Exp,
                                         scale=-0.5)
                    gtile = st1.tile([128, D], F32, name="gtile")
                    nc.vector.tensor_scalar(
                        out=gtile, in0=xt, scalar1=mv[:, 0:1], scalar2=rstd,
                        op0=mybir.AluOpType.subtract, op1=mybir.AluOpType.mult)
                    for dt in range(DT):
                        ptp = psb.tile([128, 512], F32, tag="b", name="ptp")
                        pt = ptp[:, 0:128]
                        nc.tensor.transpose(pt, gtile[:, dt * 128:(dt + 1) * 128],
                                            ident)
                        nc.scalar.copy(out=gT[:, dt, nt * 128:(nt + 1) * 128],
                                       in_=pt)

                # ---- Phases B+C: attention per head-pair ----
                for pair in range(HPAIRS):
                    wq_p = st.tile([128, DT, 256], F32R, name="wq_p")
                    nc.sync.dma_start(
                        out=wq_p, in_=wqkt_d[:, :, pair * 256:(pair + 1) * 256])
                    # QK natural for the pair: [n-tile, q64|k64|q64|k64]
                    for nt in range(NT):
                        pn = psb.tile([128, 512], F32, tag="b", name="pn")
                        for dt in range(DT):
                            nc.tensor.matmul(
                                pn[:, 0:256],
                                gT[:, dt, nt * 128:(nt + 1) * 128],
                                wq_p[:, dt, :],
                                start=(dt == 0), stop=(dt == DT - 1))
                        nc.scalar.copy(out=qkn[:, nt, :], in_=pn[:, 0:256])
                    for hh in range(2):
                        qhat = hd.tile([64, N], F32R, name="qhat")
                        khat = hd.tile([64, N], F32R, name="khat")
                        # Qt/Kt projection, heads stacked on partitions
                        for ch in range(2):
                            pp = psa.tile([128, 512], F32, tag="a", name="pp")
                            for dt in range(DT):
                                nc.tensor.matmul(
                                    pp, wq_p[:, dt, hh * 128:(hh + 1) * 128],
                                    gT[:, dt, ch * 512:(ch + 1) * 512],
                                    start=(dt == 0), stop=(dt == DT - 1))
                            nc.scalar.copy(
                                out=qhat[0:64, ch * 512:(ch + 1) * 512],
                                in_=pp[0:64, :])
                            nc.scalar.copy(
                                out=khat[0:64, ch * 512:(ch + 1) * 512],
                                in_=pp[64:128, :])
                        # orientation 1: S_qk per q-tile, exp+Z, A_kT accum
                        zcol = sm.tile([128, NT], F32, name="zcol")
                        zinvc = sm.tile([128, NT], F32, name="zinvc")
                        ak0 = psa.tile([64, 512], F32, tag="a", name="ak0")
                        ak1 = psa.tile([64, 512], F32, tag="a", name="ak1")
                        aks = (ak0, ak1)
                        for qt in range(NT):
                            e_sb = st.tile([128, N], F32R, name="e_sb")
                            z2 = sm.tile([128, 2], F32, name="z2")
                            for ch in range(2):
                                sps = psb.tile([128, 512], F32, tag="b",
                                               name="sps")
                                nc.tensor.matmul(
                                    sps, qhat[0:64, qt * 128:(qt + 1) * 128],
                                    khat[0:64, ch * 512:(ch + 1) * 512],
                                    start=True, stop=True)
                                nc.scalar.activation(
                                    out=e_sb[:, ch * 512:(ch + 1) * 512],
                                    in_=sps, func=AF.Exp, scale=BETA,
                                    bias=negc_t,
                                    accum_out=z2[:, ch:ch + 1])
                            nc.vector.reduce_sum(
                                out=zcol[:, qt:qt + 1], in_=z2,
                                axis=mybir.AxisListType.X)
                            nc.vector.reciprocal(out=zinvc[:, qt:qt + 1],
                                                 in_=zcol[:, qt:qt + 1])
                            qp = sm.tile([128, 64], F32R, name="qp")
                            nc.vector.tensor_scalar_mul(
                                out=qp, in0=qkn[:, qt, hh * 128:hh * 128 + 64],
                                scalar1=zinvc[:, qt:qt + 1])
                            for ch in range(2):
                                nc.tensor.matmul(
                                    aks[ch], qp,
                                    e_sb[:, ch * 512:(ch + 1) * 512],
                                    start=(qt == 0), stop=(qt == NT - 1))
                        for ch in range(2):
                            nc.scalar.copy(
                                out=ast[64:128, hh, ch * 512:(ch + 1) * 512],
                                in_=aks[ch])
                        # zinv row -> broadcast [64, N] for A_qT normalization
                        ltp = psb.tile([128, 512], F32, tag="b", name="ltp")
                        lt = ltp[0:NT, 0:128]
                        nc.tensor.transpose(lt, zinvc, ident)
                        zr_sb = sm.tile([NT, 128], F32R, name="zr_sb")
                        nc.scalar.copy(out=zr_sb, in_=lt)
                        zrow = sm.tile([1, N], F32R, name="zrow")
                        nc.sync.dma_start(out=zrow, in_=zr_sb)
                        zb_sb = st1.tile([64, N], F32, name="zb_sb")
                        for ch in range(2):
                            zbp = psb.tile([128, 512], F32, tag="b", name="zbp")
                            nc.tensor.matmul(
                                zbp[0:64, :], ones_c,
                                zrow[:, ch * 512:(ch + 1) * 512],
                                start=True, stop=True)
                            nc.scalar.copy(out=zb_sb[:, ch * 512:(ch + 1) * 512],
                                           in_=zbp[0:64, :])
                        # orientation 2: S_kq (65-contraction) -> P, A_qT accum
                        aq0 = psa.tile([64, 512], F32, tag="a", name="aq0")
                        aq1 = psa.tile([64, 512], F32, tag="a", name="aq1")
                        aqs = (aq0, aq1)
                        for kt in range(NT):
                            p_sb = st.tile([128, N], F32R, name="p_sb")
                            for ch in range(2):
                                s2 = psb.tile([128, 512], F32, tag="b",
                                              name="s2")
                                nc.tensor.matmul(
                                    s2, khat[:, kt * 128:(kt + 1) * 128],
                                    qhat[:, ch * 512:(ch + 1) * 512],
                                    start=True, stop=True)
                                nc.scalar.activation(
                                    out=p_sb[:, ch * 512:(ch + 1) * 512],
                                    in_=s2, func=AF.Exp, scale=BETA,
                                    bias=negc_t)
                            for ch in range(2):
                                nc.tensor.matmul(
                                    aqs[ch],
                                    qkn[:, kt, hh * 128 + 64:hh * 128 + 128],
                                    p_sb[:, ch * 512:(ch + 1) * 512],
                                    start=(kt == 0), stop=(kt == NT - 1))
                        for ch in range(2):
                            nc.vector.tensor_mul(
                                out=ast[0:64, hh, ch * 512:(ch + 1) * 512],
                                in0=aqs[ch],
                                in1=zb_sb[:, ch * 512:(ch + 1) * 512])
                    # attention gradient for this pair
                    wn_p = st.tile([128, 2, D], F32R, name="wn_p")
                    nc.sync.dma_start(
                        out=wn_p,
                        in_=WNAT[pair * 2:(pair + 1) * 2, :, :].rearrange(
                            "h p d -> p h d"))
                    for nt in range(NT):
                        gps = []
                        for ch in range(2):
                            gp = psg.tile([128, 384], F32, tag="g",
                                          name=f"gp{ch}")
                            for hh in range(2):
                                nc.tensor.matmul(
                                    gp, ast[:, hh, nt * 128:(nt + 1) * 128],
                                    wn_p[:, hh, ch * 384:(ch + 1) * 384],
                                    start=(hh == 0), stop=(hh == 1))
                            gps.append(gp)
                        for ch in range(2):
                            nc.vector.tensor_add(
                                out=x_sb[:, nt, ch * 384:(ch + 1) * 384],
                                in0=x_sb[:, nt, ch * 384:(ch + 1) * 384],
                                in1=gps[ch])

                # ---- Phase D: hopfield ----
                for mg in range(MG):
                    xin_p = st.tile([128, MGN, D], F32R, name="xin_p")
                    nc.sync.dma_start(
                        out=xin_p, in_=xin_d[:, mg * MGN:(mg + 1) * MGN, :])
                    rt = hd.tile([128, MGN, N], F32R, name="rt")
                    for mi in range(MGN):
                        m = mg * MGN + mi
                        xit_t = xitp.tile([128, DT, 128], F32R, name="xit_t")
                        nc.sync.dma_start(
                            out=xit_t, in_=xit_d[:, :, m * 128:(m + 1) * 128])
                        for ch in range(2):
                            hp = psb.tile([128, 512], F32, tag="b", name="hp")
                            for dt in range(DT):
                                nc.tensor.matmul(
                                    hp, xit_t[:, dt, :],
                                    gT[:, dt, ch * 512:(ch + 1) * 512],
                                    start=(dt == 0), stop=(dt == DT - 1))
                            nc.scalar.activation(
                                out=rt[:, mi, ch * 512:(ch + 1) * 512],
                                in_=hp, func=AF.Relu)
                    for nt in range(NT):
                        gps = []
                        for ch in range(2):
                            gp = psg.tile([128, 384], F32, tag="g",
                                          name=f"hgp{ch}")
                            for mi in range(MGN):
                                nc.tensor.matmul(
                                    gp, rt[:, mi, nt * 128:(nt + 1) * 128],
                                    xin_p[:, mi, ch * 384:(ch + 1) * 384],
                                    start=(mi == 0), stop=(mi == MGN - 1))
                            gps.append(gp)
                        for ch in range(2):
                            nc.vector.tensor_add(
                                out=x_sb[:, nt, ch * 384:(ch + 1) * 384],
                                in0=x_sb[:, nt, ch * 384:(ch + 1) * 384],
                                in1=gps[ch])

            with tc.For_i(0, steps, 1) as _i:
                step_body(_i)

            nc.sync.dma_start(out=out_d, in_=x_sb)
    nc.compile()
    return nc


@functools.lru_cache(maxsize=4)
def _get_nc(steps=STEPS):
    return _build(steps)


def kernel(x, gamma, beta_ln, Wq, Wk, xi):
    from concourse.bass_utils import run_bass_kernel_spmd

    x = np.ascontiguousarray(np.asarray(x, dtype=np.float32))
    gamma = np.asarray(gamma, dtype=np.float32)
    beta_ln = np.asarray(beta_ln, dtype=np.float32)
    Wq = np.asarray(Wq, dtype=np.float32)
    Wk = np.asarray(Wk, dtype=np.float32)
    xi = np.asarray(xi, dtype=np.float32)
    assert np.allclose(beta_ln, 0.0), "kernel assumes beta_ln == 0"

    # host-side layouts; gamma folded into the D rows of wqkt/xit
    wqkt = np.empty((D, H * 128), np.float32)
    for h in range(H):
        wqkt[:, h * 128:h * 128 + 64] = Wq[h].T
        wqkt[:, h * 128 + 64:(h + 1) * 128] = Wk[h].T
    wqkt *= gamma[:, None]
    wnat = np.concatenate([Wq, Wk], axis=1) * np.float32(ALPHA)  # [H, 128, D]
    xit = (xi.T * gamma[:, None]).copy()                          # [D, M]
    xin = (xi * np.float32(ALPHA)).copy()                         # [M, D]

    nc = _get_nc()
    in_maps = [
        {"x": np.ascontiguousarray(x[c]), "wqkt": wqkt, "wnat": wnat,
         "xit": xit, "xin": xin}
        for c in range(B)
    ]
    res = run_bass_kernel_spmd(nc, in_maps, list(range(B)))
    return np.stack([res.results[c]["out"] for c in range(B)], axis=0)


# revision 7
# speedup vs baseline: 1.0168x; 1.0168x over previous
"""Energy Transformer descent kernel for Trainium2 (8 NeuronCores).

Data-parallel over batch B=8: one batch element per core. Per core, 12 descent
steps run on-chip in a For_i loop; x stays resident in SBUF.

Per step (all matmuls fp32r = full-rate PE with ~1.8e-4 relative precision):
  g = LayerNorm(x)       (bn_stats/bn_aggr; rstd via exp(-0.5*ln(var+eps)))
  gT via PE transpose
  Per head: Qt/Kt = W @ gT (heads-stacked), S_qk per q-tile -> exp(beta*S)
  with fused row-sum Z (ACT accum_out); A_kT accumulates (Q/Z)^T @ E_qk.
  lnZ row (PE transpose + cross-partition DMA) augments Qhat row 64 so the
  65-row contraction S_kq = beta*K.Q - lnZ gives P = softmax directly;
  A_qT accumulates K^T @ P.  grad_att = [A_qT;A_kT]^T @ alpha*[Wq;Wk].
  Hopfield: H^T = (gamma*xi)^T... per m-tile -> relu -> RT; grad accumulates
  RT^T @ alpha*xi.  x += grad (alpha folded into wnat/xin on host).
gamma is folded into the D-rows of wqkt/xit on the host; beta_ln must be 0.
"""
import sys
sys.path.insert(0, "/opt/trn_rl_repo")
import functools
import numpy as np

B, N, D = 8, 1024, 768
H, Y, M = 12, 64, 3072
STEPS = 12
ALPHA = 0.1
EPS = 1e-5
BETA = 1.0 / float(np.sqrt(Y))
ESHIFT = 30.0  # exp(beta*S - ESHIFT); cancels in softmax, guards overflow
NT, DT, MT = N // 128, D // 128, M // 128   # 8, 6, 24
HPAIRS = H // 2                              # 6
MG, MGN = 8, 3                               # hopfield: 8 groups of 3 m-tiles


def _build(steps=STEPS):
    import concourse.bacc as bacc
    import concourse.mybir as mybir
    import concourse.tile as tile
    from concourse.masks import make_identity

    F32 = mybir.dt.float32
    F32R = mybir.dt.float32r
    AF = mybir.ActivationFunctionType

    import concourse.tile_utils as tile_utils
    tile_utils.max_sbuf_usage = 206 * 1024  # 224 phys / 208 usable on trn2
    nc = bacc.Bacc(None, debug=False)
    X = nc.declare_dram_parameter("x", [N, D], F32, isOutput=False)
    WQKT = nc.declare_dram_parameter("wqkt", [D, H * 128], F32R, isOutput=False)
    WNAT = nc.declare_dram_parameter("wnat", [H, 128, D], F32R, isOutput=False)
    XIT = nc.declare_dram_parameter("xit", [D, M], F32R, isOutput=False)
    XIN = nc.declare_dram_parameter("xin", [M, D], F32R, isOutput=False)
    OUT = nc.declare_dram_parameter("out", [N, D], F32, isOutput=True)

    wqkt_d = WQKT[:, :].rearrange("(t p) c -> p t c", p=128)   # [128, DT, 1536]
    xit_d = XIT[:, :].rearrange("(t p) m -> p t m", p=128)     # [128, DT, M]
    xin_d = XIN[:, :].rearrange("(t p) d -> p t d", p=128)     # [128, MT, D]
    x_d = X[:, :].rearrange("(t p) d -> p t d", p=128)         # [128, NT, D]
    out_d = OUT[:, :].rearrange("(t p) d -> p t d", p=128)

    with tile.TileContext(nc) as tc:
        with (
            tc.tile_pool(name="persist", bufs=1) as pers,
            tc.tile_pool(name="stream", bufs=2) as st,
            tc.tile_pool(name="st1", bufs=1) as st1,
            tc.tile_pool(name="xitp", bufs=3) as xitp,
            tc.tile_pool(name="heads", bufs=2) as hd,
            tc.tile_pool(name="small", bufs=4) as sm,
            tc.tile_pool(name="psb", bufs=4, space="PSUM") as psb,
            tc.tile_pool(name="psa", bufs=2, space="PSUM") as psa,
            tc.tile_pool(name="psg", bufs=2, space="PSUM") as psg,
        ):
            x_sb = pers.tile([128, NT, D], F32)
            gT = pers.tile([128, DT, N], F32R)
            ident = pers.tile([128, 128], F32)
            eps_t = pers.tile([128, 1], F32)
            negc_t = pers.tile([128, 1], F32)
            ones_row = pers.tile([1, N], F32)
            qkn = pers.tile([128, NT, 256], F32R)
            ast = pers.tile([128, 2, N], F32R)

            ones_c = pers.tile([1, 64], F32R)
            make_identity(nc, ident)
            nc.vector.memset(eps_t, EPS)
            nc.vector.memset(negc_t, -ESHIFT)
            nc.vector.memset(ones_row, 1.0)
            nc.scalar.copy(out=ones_c, in_=ones_row[0:1, 0:64])
            nc.sync.dma_start(out=x_sb, in_=x_d)

            def step_body(_iv=None):
                # ---- Phase A: LayerNorm + transpose into gT ----
                for nt in range(NT):
                    xt = x_sb[:, nt, :]
                    stats = sm.tile([128, 3, 6], F32, name="stats")
                    xg = xt.rearrange("p (s f) -> p s f", s=3)
                    for s in range(3):
                        nc.vector.bn_stats(out=stats[:, s, :], in_=xg[:, s, :])
                    mv = sm.tile([128, 2], F32, name="mv")
                    nc.vector.bn_aggr(out=mv, in_=stats)
                    lnv = sm.tile([128, 1], F32, name="lnv")
                    nc.scalar.activation(out=lnv, in_=mv[:, 1:2], func=AF.Ln,
                                         bias=eps_t)
                    rstd = sm.tile([128, 1], F32, name="rstd")
                    nc.scalar.activation(out=rstd, in_=lnv, func=AF.Exp,
                                         scale=-0.5)
                    gtile = st1.tile([128, D], F32, name="gtile")
                    nc.vector.tensor_scalar(
                        out=gtile, in0=xt, scalar1=mv[:, 0:1], scalar2=rstd,
                        op0=mybir.AluOpType.subtract, op1=mybir.AluOpType.mult)
                    for dt in range(DT):
                        ptp = psb.tile([128, 512], F32, tag="b", name="ptp")
                        pt = ptp[:, 0:128]
                        nc.tensor.transpose(pt, gtile[:, dt * 128:(dt + 1) * 128],
                                            ident)
                        nc.scalar.copy(out=gT[:, dt, nt * 128:(nt + 1) * 128],
                                       in_=pt)

                # ---- Phases B+C: attention per head-pair ----
                for pair in range(HPAIRS):
                    wq_p = st.tile([128, DT, 256], F32R, name="wq_p")
                    nc.sync.dma_start(
                        out=wq_p, in_=wqkt_d[:, :, pair * 256:(pair + 1) * 256])
                    # QK natural for the pair: [n-tile, q64|k64|q64|k64]
                    for nt in range(NT):
                        pn = psb.tile([128, 512], F32, tag="b", name="pn")
                        for dt in range(DT):
                            nc.tensor.matmul(
                                pn[:, 0:256],
                                gT[:, dt, nt * 128:(nt + 1) * 128],
                                wq_p[:, dt, :],
                                start=(dt == 0), stop=(dt == DT - 1))
                        nc.scalar.copy(out=qkn[:, nt, :], in_=pn[:, 0:256])
                    for hh in range(2):
                        qhat = hd.tile([64, N], F32R, name="qhat")
                        khat = hd.tile([64, N], F32R, name="khat")
                        # Qt/Kt projection, heads stacked on partitions
                        for ch in range(2):
                            pp = psa.tile([128, 512], F32, tag="a", name="pp")
                            for dt in range(DT):
                                nc.tensor.matmul(
                                    pp, wq_p[:, dt, hh * 128:(hh + 1) * 128],
                                    gT[:, dt, ch * 512:(ch + 1) * 512],
                                    start=(dt == 0), stop=(dt == DT - 1))
                            nc.scalar.copy(
                                out=qhat[0:64, ch * 512:(ch + 1) * 512],
                                in_=pp[0:64, :])
                            nc.scalar.copy(
                                out=khat[0:64, ch * 512:(ch + 1) * 512],
                                in_=pp[64:128, :])
                        # orientation 1: S_qk per q-tile, exp+Z, A_kT accum
                        zcol = sm.tile([128, NT], F32, name="zcol")
                        zinvc = sm.tile([128, NT], F32, name="zinvc")
                        ak0 = psa.tile([64, 512], F32, tag="a", name="ak0")
                        ak1 = psa.tile([64, 512], F32, tag="a", name="ak1")
                        aks = (ak0, ak1)
                        for qt in range(NT):
                            e_sb = st.tile([128, N], F32R, name="e_sb")
                            z2 = sm.tile([128, 2], F32, name="z2")
                            for ch in range(2):
                                sps = psb.tile([128, 512], F32, tag="b",
                                               name="sps")
                                nc.tensor.matmul(
                                    sps, qhat[0:64, qt * 128:(qt + 1) * 128],
                                    khat[0:64, ch * 512:(ch + 1) * 512],
                                    start=True, stop=True)
                                nc.scalar.activation(
                                    out=e_sb[:, ch * 512:(ch + 1) * 512],
                                    in_=sps, func=AF.Exp, scale=BETA,
                                    bias=negc_t,
                                    accum_out=z2[:, ch:ch + 1])
                            nc.vector.reduce_sum(
                                out=zcol[:, qt:qt + 1], in_=z2,
                                axis=mybir.AxisListType.X)
                            nc.vector.reciprocal(out=zinvc[:, qt:qt + 1],
                                                 in_=zcol[:, qt:qt + 1])
                            qp = sm.tile([128, 64], F32R, name="qp")
                            nc.vector.tensor_scalar_mul(
                                out=qp, in0=qkn[:, qt, hh * 128:hh * 128 + 64],
                                scalar1=zinvc[:, qt:qt + 1])
                            for ch in range(2):
                                nc.tensor.matmul(
                                    aks[ch], qp,
                                    e_sb[:, ch * 512:(ch + 1) * 512],
                                    start=(qt == 0), stop=(qt == NT - 1))
                        for ch in range(2):
                            nc.scalar.copy(
                                out=ast[64:128, hh, ch * 512:(ch + 1) * 512],
                                in_=aks[ch])
                        # zinv row -> broadcast [64, N] for A_qT normalization
                        ltp = psb.tile([128, 512], F32, tag="b", name="ltp")
                        lt = ltp[0:NT, 0:128]
                        nc.tensor.transpose(lt, zinvc, ident)
                        zr_sb = sm.tile([NT, 128], F32R, name="zr_sb")
                        nc.scalar.copy(out=zr_sb, in_=lt)
                        zrow = sm.tile([1, N], F32R, name="zrow")
                        nc.sync.dma_start(out=zrow, in_=zr_sb)
                        zb_sb = st1.tile([64, N], F32, name="zb_sb")
                        for ch in range(2):
                            zbp = psb.tile([128, 512], F32, tag="b", name="zbp")
                            nc.tensor.matmul(
                                zbp[0:64, :], ones_c,
                                zrow[:, ch * 512:(ch + 1) * 512],
                                start=True, stop=True)
                            nc.scalar.copy(out=zb_sb[:, ch * 512:(ch + 1) * 512],
                                           in_=zbp[0:64, :])
                        # orientation 2: S_kq (65-contraction) -> P, A_qT accum
                        aq0 = psa.tile([64, 512], F32, tag="a", name="aq0")
                        aq1 = psa.tile([64, 512], F32, tag="a", name="aq1")
                        aqs = (aq0, aq1)
                        for kt in range(NT):
                            p_sb = st.tile([128, N], F32R, name="p_sb")
                            for ch in range(2):
                                s2 = psb.tile([128, 512], F32, tag="b",
                                              name="s2")
                                nc.tensor.matmul(
                                    s2, khat[:, kt * 128:(kt + 1) * 128],
                                    qhat[:, ch * 512:(ch + 1) * 512],
                                    start=True, stop=True)
                                nc.scalar.activation(
                                    out=p_sb[:, ch * 512:(ch + 1) * 512],
                                    in_=s2, func=AF.Exp, scale=BETA,
                                    bias=negc_t)
                            for ch in range(2):
                                nc.tensor.matmul(
                                    aqs[ch],
                                    qkn[:, kt, hh * 128 + 64:hh * 128 + 128],
                                    p_sb[:, ch * 512:(ch + 1) * 512],
                                    start=(kt == 0), stop=(kt == NT - 1))
                        for ch in range(2):
                            nc.vector.tensor_mul(
                                out=ast[0:64, hh, ch * 512:(ch + 1) * 512],
                                in0=aqs[ch],
                                in1=zb_sb[:, ch * 512:(ch + 1) * 512])
                    # attention gradient for this pair
                    wn_p = st.tile([128, 2, D], F32R, name="wn_p")
                    nc.sync.dma_start(
                        out=wn_p,
                        in_=WNAT[pair * 2:(pair + 1) * 2, :, :].rearrange(
                            "h p d -> p h d"))
                    for nt in range(NT):
                        gps = []
                        for ch in range(2):
                            gp = psg.tile([128, 384], F32, tag="g",
                                          name=f"gp{ch}")
                            for hh in range(2):
                                nc.tensor.matmul(
                                    gp, ast[:, hh, nt * 128:(nt + 1) * 128],
                                    wn_p[:, hh, ch * 384:(ch + 1) * 384],
                                    start=(hh == 0), stop=(hh == 1))
                            gps.append(gp)
                        for ch in range(2):
                            nc.vector.tensor_add(
                                out=x_sb[:, nt, ch * 384:(ch + 1) * 384],
                                in0=x_sb[:, nt, ch * 384:(ch + 1) * 384],
                                in1=gps[ch])

                # ---- Phase D: hopfield ----
                for mg in range(MG):
                    xin_p = st.tile([128, MGN, D], F32R, name="xin_p")
                    nc.sync.dma_start(
                        out=xin_p, in_=xin_d[:, mg * MGN:(mg + 1) * MGN, :])
                    rt = hd.tile([128, MGN, N], F32R, name="rt")
                    for mi in range(MGN):
                        m = mg * MGN + mi
                        xit_t = xitp.tile([128, DT, 128], F32R, name="xit_t")
                        nc.sync.dma_start(
                            out=xit_t, in_=xit_d[:, :, m * 128:(m + 1) * 128])
                        for ch in range(2):
                            hp = psb.tile([128, 512], F32, tag="b", name="hp")
                            for dt in range(DT):
                                nc.tensor.matmul(
                                    hp, xit_t[:, dt, :],
                                    gT[:, dt, ch * 512:(ch + 1) * 512],
                                    start=(dt == 0), stop=(dt == DT - 1))
                            nc.scalar.activation(
                                out=rt[:, mi, ch * 512:(ch + 1) * 512],
                                in_=hp, func=AF.Relu)
                    for nt in range(NT):
                        gps = []
                        for ch in range(2):
                            gp = psg.tile([128, 384], F32, tag="g",
                                          name=f"hgp{ch}")
                            for mi in range(MGN):
                                nc.tensor.matmul(
                                    gp, rt[:, mi, nt * 128:(nt + 1) * 128],
                                    xin_p[:, mi, ch * 384:(ch + 1) * 384],
                                    start=(mi == 0), stop=(mi == MGN - 1))
                            gps.append(gp)
                        for ch in range(2):
                            nc.vector.tensor_add(
                                out=x_sb[:, nt, ch * 384:(ch + 1) * 384],
                                in0=x_sb[:, nt, ch * 384:(ch + 1) * 384],
                                in1=gps[ch])

            with tc.For_i(0, steps, 1) as _i:
                step_body(_i)

            nc.sync.dma_start(out=out_d, in_=x_sb)
    nc.compile()
    return nc


@functools.lru_cache(maxsize=4)
def _get_nc(steps=STEPS):
    return _build(steps)


def kernel(x, gamma, beta_ln, Wq, Wk, xi):
    from concourse.bass_utils import run_bass_kernel_spmd

    x = np.ascontiguousarray(np.asarray(x, dtype=np.float32))
    gamma = np.asarray(gamma, dtype=np.float32)
    beta_ln = np.asarray(beta_ln, dtype=np.float32)
    Wq = np.asarray(Wq, dtype=np.float32)
    Wk = np.asarray(Wk, dtype=np.float32)
    xi = np.asarray(xi, dtype=np.float32)
    assert np.allclose(beta_ln, 0.0), "kernel assumes beta_ln == 0"

    # host-side layouts; gamma folded into the D rows of wqkt/xit
    wqkt = np.empty((D, H * 128), np.float32)
    for h in range(H):
        wqkt[:, h * 128:h * 128 + 64] = Wq[h].T
        wqkt[:, h * 128 + 64:(h + 1) * 128] = Wk[h].T
    wqkt *= gamma[:, None]
    wnat = np.concatenate([Wq, Wk], axis=1) * np.float32(ALPHA)  # [H, 128, D]
    xit = (xi.T * gamma[:, None]).copy()                          # [D, M]
    xin = (xi * np.float32(ALPHA)).copy()                         # [M, D]

    nc = _get_nc()
    in_maps = [
        {"x": np.ascontiguousarray(x[c]), "wqkt": wqkt, "wnat": wnat,
         "xit": xit, "xin": xin}
        for c in range(B)
    ]
    res = run_bass_kernel_spmd(nc, in_maps, list(range(B)))
    return np.stack([res.results[c]["out"] for c in range(B)], axis=0)
